# revision 38
# baseline (speedup 1.0000x reference)
"""Multi-head attention (B=2, S=2048, D=1024, H=16, causal + key-padding mask)
for 8 Trainium2 NeuronCores.

Sharding: data + head parallel. Core c handles batch b = c//4 and the 4 heads
h in [4*(c%4), 4*(c%4)+4). Q/K/V/O projection weights are column/row-sliced
per core (Megatron style); the output projection partial sums are reduced on
the host (4 cores per batch), which also applies the output bias.

Data tier: fp16 (full PE rate, 10-bit mantissa -> ~1e-3 scale-relative
error); all accumulation fp32 in PSUM, softmax exp/normalization arithmetic
fp32 internally. The softmax numerator and denominator both come from the
same fp16 exp tile, so short-row quantization errors cancel.

Per-core layouts (all DMAs contiguous):
  qT, kT [128, 2048] per head-pair (partition = 2x64 head dims); v with an
  appended ones column [128k, 4h, 65]; scoresT [k, q] blocks of [128, 512]
  computed two-at-a-time into one [128, 1024] PSUM pair so each ACT exp
  covers 1024 columns; the ones column makes row 64 of the attn@V PSUM the
  softmax denominator for free. Causal masking accumulates (-30000*I) @
  mask01[r] into diagonal score blocks before exp; the key-padding mask is
  the per-partition exp bias. The two heads of a pair are interleaved at
  base partitions 0/64 so the K=64 score matmuls pack into disjoint PE
  row-groups. reciprocal_approx_fast + a K=1 ones matmul broadcasts the
  reciprocal sums across partitions for the final normalize.
"""

import os

import numpy as np

import concourse.tile as tile
import concourse.mybir as mybir
from concourse import bacc
from concourse.bass_utils import run_bass_kernel_spmd

F32 = mybir.dt.float32
F16 = mybir.dt.float16
AF = mybir.ActivationFunctionType
MUL = mybir.AluOpType.mult
ADD = mybir.AluOpType.add

B, S, D, H = 2, 2048, 1024, 16
HD = D // H            # 64 head dim
NCORES = 8
HPC = H // (NCORES // B)   # 4 heads per core
NJ = S // 128          # 16 k-tiles of 128
NG = S // 512          # 4 q-groups of 512
NDT = D // 128         # 8 d-tiles of the model dim
NEG = -30000.0         # fp16-representable; exp(s + NEG) == 0
NEGPAD = -1.0e30       # fp32 bias for padded keys

_CACHED = {}


def _build():
    nc = bacc.Bacc("TRN2", target_bir_lowering=False, debug=False,
                   num_devices=NCORES)

    qt = nc.dram_tensor("qt", [128, NDT, S], F16, kind="ExternalInput").ap()
    kt = nc.dram_tensor("kt", [128, NDT, S], F16, kind="ExternalInput").ap()
    vt = nc.dram_tensor("vt", [128, NDT, S], F16, kind="ExternalInput").ap()
    wqt = nc.dram_tensor("wqt", [128, NDT, 256], F16, kind="ExternalInput").ap()
    wkt = nc.dram_tensor("wkt", [128, NDT, 256], F16, kind="ExternalInput").ap()
    wvt = nc.dram_tensor("wvt", [128, NDT, 256], F16, kind="ExternalInput").ap()
    wot = nc.dram_tensor("wot", [128, 2, D], F16, kind="ExternalInput").ap()
    bq = nc.dram_tensor("bq", [128, 2], F32, kind="ExternalInput").ap()
    bk = nc.dram_tensor("bk", [128, 2], F32, kind="ExternalInput").ap()
    bvb = nc.dram_tensor("bvb", [128, 256], F32, kind="ExternalInput").ap()
    padb = nc.dram_tensor("padb", [128, NJ], F32, kind="ExternalInput").ap()
    masksq = nc.dram_tensor("masksq", [128, 128], F16, kind="ExternalInput").ap()
    negi = nc.dram_tensor("negi", [128, 128], F16, kind="ExternalInput").ap()

    attnt = nc.dram_tensor("attnt", [HPC, S, S], F16, kind="ExternalOutput").ap()
    outpt = nc.dram_tensor("outpt", [D, S], F32, kind="ExternalOutput").ap()

    with tile.TileContext(nc) as tc:
        with (
            tc.tile_pool(name="consts", bufs=1) as consts,
            tc.tile_pool(name="persist", bufs=1) as persist,
            tc.tile_pool(name="ps", bufs=2, space="PSUM") as ps,
            tc.tile_pool(name="po", bufs=2, space="PSUM") as po,
        ):
            # ---- constants ----
            ones_row = consts.tile([1, 128], F32)
            nc.vector.memset(ones_row[:], 1.0)
            ones_row_h = consts.tile([1, 128], F16)
            nc.vector.tensor_copy(ones_row_h[:], ones_row[:])
            ones_4 = consts.tile([128, HPC, 1], F32)
            nc.vector.memset(ones_4[:], 1.0)
            masksq_sb = consts.tile([128, 128], F16)
            nc.sync.dma_start(masksq_sb[:], masksq[:])
            negi_sb = consts.tile([128, 128], F16)
            nc.sync.dma_start(negi_sb[:], negi[:])
            padb_sb = consts.tile([128, NJ], F32)
            nc.sync.dma_start(padb_sb[:], padb[:])
            bq_sb = consts.tile([128, 2], F32)
            nc.sync.dma_start(bq_sb[:], bq[:])
            bk_sb = consts.tile([128, 2], F32)
            nc.sync.dma_start(bk_sb[:], bk[:])
            bvb_sb = consts.tile([128, 256], F32)
            nc.sync.dma_start(bvb_sb[:], bvb[:])
            wo_sb = consts.tile([128, 2, D], F16)
            nc.sync.dma_start(wo_sb[:], wot[:])

            # ---- persistent activations ----
            qT_sb = [persist.tile([128, S], F16, name=f"qT{p}") for p in range(2)]
            kT_sb = [persist.tile([128, S], F16, name=f"kT{p}") for p in range(2)]
            v_sb = [persist.tile([128, HPC, HD + 1], F16, name=f"v{j}")
                    for j in range(NJ)]
            oT_sb = [persist.tile([128, S], F16, name=f"oT{p}") for p in range(2)]

            # ---- projections ----
            with (
                tc.tile_pool(name="inp", bufs=9) as inp,
                tc.tile_pool(name="wts", bufs=1) as wts,
            ):
                # V projection: v[s, d'] = sum_d VT[d, s] * wvT[d, d'] + bv.
                # d-outer with 4 interleaved PSUM chains per round so the PE
                # starts as soon as the first d-tile lands. Chains get
                # bank-exclusive [128, 256] slices; rounds alternate between
                # the ps and po pools (po is idle during projections).
                wv_sb = wts.tile([128, NDT, 256], F16)
                nc.sync.dma_start(wv_sb[:], wvt[:])
                vtiles = [inp.tile([128, S], F16, tag="inp", name=f"vt{dt}",
                                   uniquify=True)
                          for dt in range(NDT)]
                # chunked loads in compute order: round-major so round 0's
                # matmuls start as soon as its 8 chunks land
                for rnd in range(4):
                    for dt in range(NDT):
                        nc.sync.dma_start(
                            vtiles[dt][:, rnd * 512:(rnd + 1) * 512],
                            vt[:, dt, rnd * 512:(rnd + 1) * 512],
                        )
                for rnd in range(4):
                    pool = ps if rnd % 2 == 0 else po
                    tag = "mm" if rnd % 2 == 0 else "o"
                    tiles = [pool.tile([128, 1024], F32, tag=tag,
                                       name=f"pv{rnd}{t}") for t in range(2)]
                    slots = [tiles[0][:, :256], tiles[0][:, 512:768],
                             tiles[1][:, :256], tiles[1][:, 512:768]]
                    for dt in range(NDT):
                        for q in range(4):
                            st = 4 * rnd + q
                            nc.tensor.matmul(
                                slots[q],
                                vtiles[dt][:, st * 128:(st + 1) * 128],
                                wv_sb[:, dt],
                                start=(dt == 0), stop=(dt == NDT - 1),
                            )
                    for q in range(4):
                        st = 4 * rnd + q
                        nc.vector.tensor_tensor(
                            v_sb[st][:, :, :HD],
                            slots[q].rearrange("p (h d) -> p h d", h=HPC),
                            bvb_sb.rearrange("p (h d) -> p h d", h=HPC),
                            ADD,
                        )
                        nc.vector.tensor_copy(
                            v_sb[st][:, :, HD:HD + 1], ones_4[:]
                        )

                # Q/K projections: xT[d', s] = sum_d wxT[d, d'] * XT[d, s] + bx
                wq_sb = wts.tile([128, NDT, 256], F16)
                nc.sync.dma_start(wq_sb[:], wqt[:])
                wk_sb = wts.tile([128, NDT, 256], F16)
                nc.sync.dma_start(wk_sb[:], wkt[:])
                for which, wsb, xdram, bsb, dst in (
                    ("q", wq_sb, qt, bq_sb, qT_sb),
                    ("k", wk_sb, kt, bk_sb, kT_sb),
                ):
                    xtiles = [inp.tile([128, S], F16, tag="inp",
                                       name=f"{which}t{dt}")
                              for dt in range(NDT)]
                    for dt in range(NDT):
                        for gc in range(NG):
                            nc.sync.dma_start(
                                xtiles[dt][:, gc * 512:(gc + 1) * 512],
                                xdram[:, dt, gc * 512:(gc + 1) * 512],
                            )
                    for pair in range(2):
                        pool = ps if pair == 0 else po
                        tag = "mm" if pair == 0 else "o"
                        tiles = [pool.tile([128, 1024], F32, tag=tag,
                                           name=f"px{pair}{t}")
                                 for t in range(2)]
                        slots = [tiles[0][:, :512], tiles[0][:, 512:],
                                 tiles[1][:, :512], tiles[1][:, 512:]]
                        for dt in range(NDT):
                            for g in range(NG):
                                nc.tensor.matmul(
                                    slots[g],
                                    wsb[:, dt, pair * 128:(pair + 1) * 128],
                                    xtiles[dt][:, g * 512:(g + 1) * 512],
                                    start=(dt == 0), stop=(dt == NDT - 1),
                                )
                        for g in range(NG):
                            nc.scalar.activation(
                                dst[pair][:, g * 512:(g + 1) * 512],
                                slots[g],
                                AF.Identity, bias=bsb[:, pair:pair + 1],
                            )

            # ---- attention main loop ----
            with (
                tc.tile_pool(name="exps", bufs=2 * NJ + 2) as expp,
                tc.tile_pool(name="small", bufs=4) as small,
                tc.tile_pool(name="astage", bufs=6) as astage,
            ):
                def emit_normalize(g, pair, exps, psum_o):
                    J = 4 * g + 4
                    # late iterations run while ACT is mostly idle; early ones
                    # while ACT is saturated with exps -> pick evac engine
                    evac = nc.scalar.copy if g <= 1 else nc.vector.tensor_copy
                    rbs = {}
                    # both reciprocal chains first so neither's broadcast
                    # matmul queues behind the other's normalize TTs
                    for sub in range(2):
                        s_f = small.tile([1, 512], F32, tag="sf", name="s_f")
                        nc.vector.tensor_copy(s_f[:], psum_o[sub][HD:HD + 1, :])
                        r_f = small.tile([1, 512], F32, tag="rf", name="r_f")
                        nc.vector.reciprocal_approx_fast(r_f[:], s_f[:])
                        r_h = small.tile([1, 512], F16, tag="rh", name="r_h")
                        nc.vector.tensor_copy(r_h[:], r_f[:])
                        prb = ps.tile([128, 1024], F32, tag="mm", name="prb")
                        nc.tensor.matmul(
                            prb[:, :512], ones_row_h[:], r_h[:],
                            start=True, stop=True,
                        )
                        rb_sb = small.tile([128, 512], F16, tag="rb",
                                           name="rb_sb")
                        evac(rb_sb[:], prb[:, :512])
                        rbs[sub] = rb_sb
                    for sub in range(2):
                        off = 64 * sub
                        rb_sb = rbs[sub]
                        h = 2 * pair + sub
                        for j, e in enumerate(exps):
                            r = j - 4 * g
                            w0 = 128 * r if r > 0 else 0
                            a = astage.tile([128, 512], F16, tag="a", name="a")
                            nc.vector.tensor_tensor(
                                a[:, w0:],
                                e[:, sub * 512 + w0:(sub + 1) * 512],
                                rb_sb[:, w0:], MUL,
                            )
                            nc.sync.dma_start(
                                attnt[h, j * 128:(j + 1) * 128,
                                      g * 512 + w0:(g + 1) * 512],
                                a[:, w0:],
                            )
                        nc.vector.tensor_tensor(
                            oT_sb[pair][off:off + 64, g * 512:(g + 1) * 512],
                            psum_o[sub][:HD, :], rbs[sub][:HD, :], MUL,
                        )

                def emit_outproj(g):
                    evac = nc.scalar.copy if g <= 1 else nc.vector.tensor_copy
                    for mt in range(NDT):
                        pp = ps.tile([128, 1024], F32, tag="mm", name="pp")
                        for dt in range(2):
                            nc.tensor.matmul(
                                pp[:, :512],
                                wo_sb[:, dt, mt * 128:(mt + 1) * 128],
                                oT_sb[dt][:, g * 512:(g + 1) * 512],
                                start=(dt == 0), stop=(dt == 1),
                            )
                        o = astage.tile([128, 512], F32, tag="of", name="o")
                        evac(o[:], pp[:, :512])
                        nc.sync.dma_start(
                            outpt[mt * 128:(mt + 1) * 128,
                                  g * 512:(g + 1) * 512],
                            o[:],
                        )

                pending = None  # (g, pair, exps, psum_o) awaiting normalize
                for g in reversed(range(NG)):
                    J = 4 * g + 4
                    for pair in range(2):
                        exps = []  # e[j]: [128, 1024] = (sub0 | sub1) halves
                        pot = po.tile([128, 1024], F32, tag="o", name="pot")
                        psum_o = {
                            sub: pot[:HD + 1, 512 * sub:512 * sub + 512]
                            for sub in range(2)
                        }
                        for j0 in range(0, J, 2):
                            # each [128, 1024] PSUM pair-tile holds one k-tile
                            # j for BOTH heads of the pair (sub0 | sub1), so a
                            # single exp covers both with the right per-j
                            # padding bias. Consecutive score matmuls
                            # alternate base partitions 0/64 -> disjoint PE
                            # row groups run concurrently.
                            pss = {
                                jj: ps.tile([128, 1024], F32, tag="mm",
                                            name=f"pss{jj}")
                                for jj in range(2)
                            }
                            for jj in range(2):
                                j = j0 + jj
                                r = j - 4 * g
                                for sub in range(2):
                                    off = 64 * sub
                                    nc.tensor.matmul(
                                        pss[jj][:, sub * 512:(sub + 1) * 512],
                                        kT_sb[pair][off:off + 64,
                                                    j * 128:(j + 1) * 128],
                                        qT_sb[pair][off:off + 64,
                                                    g * 512:(g + 1) * 512],
                                        start=True, stop=(r < 0),
                                    )
                                if r >= 0:
                                    # staircase mask covers only the partial
                                    # 128-wide diagonal strip; the fully
                                    # masked columns below are never exp'd
                                    for sub in range(2):
                                        nc.tensor.matmul(
                                            pss[jj][:, sub * 512 + 128 * r:
                                                    sub * 512 + 128 * (r + 1)],
                                            negi_sb[:],
                                            masksq_sb[:],
                                            start=False, stop=True,
                                        )
                            for jj in range(2):
                                j = j0 + jj
                                r = j - 4 * g
                                w0 = 128 * r if r > 0 else 0
                                e = expp.tile([128, 1024], F16, tag="exp",
                                              name="e")
                                if w0 == 0:
                                    nc.scalar.activation(
                                        e[:], pss[jj][:], AF.Exp,
                                        bias=padb_sb[:, j:j + 1],
                                    )
                                else:
                                    for sub in range(2):
                                        nc.gpsimd.memset(
                                            e[:, sub * 512:sub * 512 + w0], 0.0
                                        )
                                        nc.scalar.activation(
                                            e[:, sub * 512 + w0:
                                              (sub + 1) * 512],
                                            pss[jj][:, sub * 512 + w0:
                                                    (sub + 1) * 512],
                                            AF.Exp,
                                            bias=padb_sb[:, j:j + 1],
                                        )
                                exps.append(e)
                            for jj in range(2):
                                j = j0 + jj
                                e = exps[j]
                                for sub in range(2):
                                    h = 2 * pair + sub
                                    nc.tensor.matmul(
                                        psum_o[sub][:],
                                        v_sb[j][:, h],
                                        e[:, sub * 512:(sub + 1) * 512],
                                        start=(j == 0), stop=(j == J - 1),
                                    )
                        # normalize of the PREVIOUS iteration lands here, so
                        # its reciprocal chain overlaps this iteration's
                        # matmul stream instead of stalling the PE
                        if pending is not None:
                            emit_normalize(*pending)
                            if pending[1] == 1:
                                emit_outproj(pending[0])
                        pending = (g, pair, exps, psum_o)
                emit_normalize(*pending)
                emit_outproj(pending[0])

    nc.compile()
    return nc


def _rearr_dxs(x, dtype=np.float16):
    # [Dm, S] -> [128, Dm//128, S] contiguous (partition-major d-tiles)
    return np.ascontiguousarray(
        x.reshape(x.shape[0] // 128, 128, x.shape[1]).transpose(1, 0, 2)
    ).astype(dtype)


def kernel(Q, K, V, attention_mask, wq, bq, wk, bk, wv, bv, wo, bo):
    Q = np.asarray(Q, np.float32)
    K = np.asarray(K, np.float32)
    V = np.asarray(V, np.float32)
    attention_mask = np.asarray(attention_mask)
    wq, bq_, wk, bk_ = (np.asarray(a, np.float32) for a in (wq, bq, wk, bk))
    wv, bv_, wo, bo_ = (np.asarray(a, np.float32) for a in (wv, bv, wo, bo))

    if "nc" not in _CACHED:
        _CACHED["nc"] = _build()
    nc = _CACHED["nc"]

    scale = 1.0 / np.sqrt(np.float32(HD))

    p = np.arange(128)[:, None]
    f = np.arange(128)[None, :]
    masksq = (f < p).astype(np.float16)  # invalid iff f' < p in diag strip
    negi = (NEG * np.eye(128)).astype(np.float16)

    in_maps = []
    for c in range(NCORES):
        b = c // (NCORES // B)
        hg = c % (NCORES // B)
        sl = slice(hg * HPC * HD, (hg + 1) * HPC * HD)  # this core's 256 dims

        padbias = np.where(attention_mask[b] != 0, 0.0, NEGPAD).astype(np.float32)
        in_maps.append({
            "qt": _rearr_dxs(Q[b].T),
            "kt": _rearr_dxs(K[b].T),
            "vt": _rearr_dxs(V[b].T),
            "wqt": _rearr_dxs(np.ascontiguousarray(wq.T[:, sl])),
            "wkt": _rearr_dxs(np.ascontiguousarray(wk.T[:, sl] * scale)),
            "wvt": _rearr_dxs(np.ascontiguousarray(wv.T[:, sl])),
            "wot": _rearr_dxs(np.ascontiguousarray(wo.T[sl, :])),
            "bq": np.ascontiguousarray(bq_[sl].reshape(2, 128).T),
            "bk": np.ascontiguousarray((bk_[sl] * scale).reshape(2, 128).T),
            "bvb": np.broadcast_to(bv_[sl], (128, 256)).copy(),
            "padb": np.ascontiguousarray(padbias.reshape(NJ, 128).T),
            "masksq": masksq,
            "negi": negi,
        })

    trace = bool(os.environ.get("MHA_TRACE"))
    res = run_bass_kernel_spmd(
        nc, in_maps, core_ids=list(range(NCORES)), trace=trace
    )
    if trace:
        kernel.last_exec_time_ns = res.exec_time_ns
        kernel.last_trace = (
            res.instructions_and_trace[1] if res.instructions_and_trace else None
        )

    # ---- host gather ----
    out = np.zeros((B, S, D), np.float32)
    attn = np.zeros((B, H, S, S), np.float32)
    tril = np.tril(np.ones((S, S), bool))
    for c in range(NCORES):
        b = c // (NCORES // B)
        hg = c % (NCORES // B)
        rc = res.results[c]
        out[b] += rc["outpt"].T
        for hl in range(HPC):
            h = hg * HPC + hl
            attn[b, h] = np.where(tril, rc["attnt"][hl].astype(np.float32).T, 0.0)
    out += bo_[None, None, :]
    return out, attn


# revision 46
# speedup vs baseline: 1.0621x; 1.0621x over previous
"""Multi-head attention (B=2, S=2048, D=1024, H=16, causal + key-padding mask)
for 8 Trainium2 NeuronCores.

Sharding: data + head parallel. Core c handles batch b = c//4 and the 4 heads
h in [4*(c%4), 4*(c%4)+4). Q/K/V/O projection weights are column/row-sliced
per core (Megatron style); the output projection partial sums are reduced on
the host (4 cores per batch), which also applies the output bias.

Data tier: fp16 (full PE rate, 10-bit mantissa -> ~1e-3 scale-relative
error); all accumulation fp32 in PSUM, softmax exp/normalization arithmetic
fp32 internally. The softmax numerator and denominator both come from the
same fp16 exp tile, so short-row quantization errors cancel.

Per-core layouts (all DMAs contiguous):
  qT, kT [128, 2048] per head-pair (partition = 2x64 head dims); v with an
  appended ones column [128k, 4h, 65]; scoresT [k, q] blocks of [128, 512]
  computed two-at-a-time into one [128, 1024] PSUM pair so each ACT exp
  covers 1024 columns; the ones column makes row 64 of the attn@V PSUM the
  softmax denominator for free. Causal masking accumulates (-30000*I) @
  mask01[r] into diagonal score blocks before exp; the key-padding mask is
  the per-partition exp bias. The two heads of a pair are interleaved at
  base partitions 0/64 so the K=64 score matmuls pack into disjoint PE
  row-groups. reciprocal_approx_fast + a K=1 ones matmul broadcasts the
  reciprocal sums across partitions for the final normalize.
"""

import os

import numpy as np

import concourse.tile as tile
import concourse.mybir as mybir
from concourse import bacc
from concourse.bass_utils import run_bass_kernel_spmd

F32 = mybir.dt.float32
F16 = mybir.dt.float16
AF = mybir.ActivationFunctionType
MUL = mybir.AluOpType.mult
ADD = mybir.AluOpType.add

B, S, D, H = 2, 2048, 1024, 16
HD = D // H            # 64 head dim
NCORES = 8
HPC = H // (NCORES // B)   # 4 heads per core
NJ = S // 128          # 16 k-tiles of 128
NG = S // 512          # 4 q-groups of 512
NDT = D // 128         # 8 d-tiles of the model dim
NEG = -30000.0         # fp16-representable; exp(s + NEG) == 0
NEGPAD = -1.0e30       # fp32 bias for padded keys

_CACHED = {}


def _build():
    nc = bacc.Bacc("TRN2", target_bir_lowering=False, debug=False,
                   num_devices=NCORES)

    qt = nc.dram_tensor("qt", [128, NDT, S], F16, kind="ExternalInput").ap()
    kt = nc.dram_tensor("kt", [128, NDT, S], F16, kind="ExternalInput").ap()
    vt = nc.dram_tensor("vt", [128, NDT, S], F16, kind="ExternalInput").ap()
    wqt = nc.dram_tensor("wqt", [128, NDT, 256], F16, kind="ExternalInput").ap()
    wkt = nc.dram_tensor("wkt", [128, NDT, 256], F16, kind="ExternalInput").ap()
    wvt = nc.dram_tensor("wvt", [128, NDT, 256], F16, kind="ExternalInput").ap()
    wot = nc.dram_tensor("wot", [128, 2, D], F16, kind="ExternalInput").ap()
    bq = nc.dram_tensor("bq", [128, 2], F32, kind="ExternalInput").ap()
    bk = nc.dram_tensor("bk", [128, 2], F32, kind="ExternalInput").ap()
    bvb = nc.dram_tensor("bvb", [128, 256], F32, kind="ExternalInput").ap()
    padb = nc.dram_tensor("padb", [128, NJ], F32, kind="ExternalInput").ap()
    masksq = nc.dram_tensor("masksq", [128, 128], F16, kind="ExternalInput").ap()
    negi = nc.dram_tensor("negi", [128, 128], F16, kind="ExternalInput").ap()

    attnt = nc.dram_tensor("attnt", [HPC, S, S], F16, kind="ExternalOutput").ap()
    outpt = nc.dram_tensor("outpt", [D, S], F32, kind="ExternalOutput").ap()

    with tile.TileContext(nc) as tc:
        with (
            tc.tile_pool(name="consts", bufs=1) as consts,
            tc.tile_pool(name="persist", bufs=1) as persist,
            tc.tile_pool(name="ps", bufs=2, space="PSUM") as ps,
            tc.tile_pool(name="po", bufs=2, space="PSUM") as po,
        ):
            # ---- constants ----
            ones_row = consts.tile([1, 128], F32)
            nc.vector.memset(ones_row[:], 1.0)
            ones_row_h = consts.tile([1, 128], F16)
            nc.vector.tensor_copy(ones_row_h[:], ones_row[:])
            ones_4 = consts.tile([128, HPC, 1], F32)
            nc.vector.memset(ones_4[:], 1.0)
            masksq_sb = consts.tile([128, 128], F16)
            nc.sync.dma_start(masksq_sb[:], masksq[:])
            negi_sb = consts.tile([128, 128], F16)
            nc.sync.dma_start(negi_sb[:], negi[:])
            padb_sb = consts.tile([128, NJ], F32)
            nc.sync.dma_start(padb_sb[:], padb[:])
            bq_sb = consts.tile([128, 2], F32)
            nc.sync.dma_start(bq_sb[:], bq[:])
            bk_sb = consts.tile([128, 2], F32)
            nc.sync.dma_start(bk_sb[:], bk[:])
            bvb_sb = consts.tile([128, 256], F32)
            nc.sync.dma_start(bvb_sb[:], bvb[:])
            wo_sb = consts.tile([128, 2, D], F16)
            nc.sync.dma_start(wo_sb[:], wot[:])

            # ---- persistent activations ----
            qT_sb = [persist.tile([128, S], F16, name=f"qT{p}") for p in range(2)]
            kT_sb = [persist.tile([128, S], F16, name=f"kT{p}") for p in range(2)]
            v_sb = [persist.tile([128, HPC, HD + 1], F16, name=f"v{j}")
                    for j in range(NJ)]
            oT_sb = [persist.tile([128, S], F16, name=f"oT{p}") for p in range(2)]

            # ---- projections ----
            with (
                tc.tile_pool(name="inp", bufs=3) as inp,
                tc.tile_pool(name="wts", bufs=1) as wts,
            ):
                # V projection: v[s, d'] = sum_d VT[d, s] * wvT[d, d'] + bv.
                # d-outer with 4 interleaved PSUM chains per round so the PE
                # starts as soon as the first d-tile lands. Chains get
                # bank-exclusive [128, 256] slices; rounds alternate between
                # the ps and po pools (po is idle during projections).
                wv_sb = wts.tile([128, NDT, 256], F16)
                nc.sync.dma_start(wv_sb[:], wvt[:])
                # two 2-MiB DMAs per input tensor: each dma_start fans out
                # across all 16 SDMA engines, and fewer DMAs means less
                # serialized issue/completion overhead
                vhalves = [inp.tile([128, NDT // 2, S], F16, tag="inp",
                                    name=f"vth{hh}") for hh in range(2)]
                for hh in range(2):
                    nc.sync.dma_start(
                        vhalves[hh][:], vt[:, hh * 4:(hh + 1) * 4, :]
                    )
                vtiles = [vhalves[dt // 4][:, dt % 4] for dt in range(NDT)]
                for rnd in range(4):
                    pool = ps if rnd % 2 == 0 else po
                    tag = "mm" if rnd % 2 == 0 else "o"
                    tiles = [pool.tile([128, 1024], F32, tag=tag,
                                       name=f"pv{rnd}{t}") for t in range(2)]
                    slots = [tiles[0][:, :256], tiles[0][:, 512:768],
                             tiles[1][:, :256], tiles[1][:, 512:768]]
                    for dt in range(NDT):
                        for q in range(4):
                            st = 4 * rnd + q
                            nc.tensor.matmul(
                                slots[q],
                                vtiles[dt][:, st * 128:(st + 1) * 128],
                                wv_sb[:, dt],
                                start=(dt == 0), stop=(dt == NDT - 1),
                            )
                    for q in range(4):
                        st = 4 * rnd + q
                        nc.vector.tensor_tensor(
                            v_sb[st][:, :, :HD],
                            slots[q].rearrange("p (h d) -> p h d", h=HPC),
                            bvb_sb.rearrange("p (h d) -> p h d", h=HPC),
                            ADD,
                        )
                        nc.vector.tensor_copy(
                            v_sb[st][:, :, HD:HD + 1], ones_4[:]
                        )

                # Q/K projections: xT[d', s] = sum_d wxT[d, d'] * XT[d, s] + bx
                wq_sb = wts.tile([128, NDT, 256], F16)
                nc.sync.dma_start(wq_sb[:], wqt[:])
                wk_sb = wts.tile([128, NDT, 256], F16)
                nc.sync.dma_start(wk_sb[:], wkt[:])
                for which, wsb, xdram, bsb, dst in (
                    ("q", wq_sb, qt, bq_sb, qT_sb),
                    ("k", wk_sb, kt, bk_sb, kT_sb),
                ):
                    xhalves = [inp.tile([128, NDT // 2, S], F16, tag="inp",
                                        name=f"{which}th{hh}")
                               for hh in range(2)]
                    for hh in range(2):
                        nc.sync.dma_start(
                            xhalves[hh][:], xdram[:, hh * 4:(hh + 1) * 4, :]
                        )
                    xtiles = [xhalves[dt // 4][:, dt % 4] for dt in range(NDT)]
                    for pair in range(2):
                        pool = ps if pair == 0 else po
                        tag = "mm" if pair == 0 else "o"
                        tiles = [pool.tile([128, 1024], F32, tag=tag,
                                           name=f"px{pair}{t}")
                                 for t in range(2)]
                        slots = [tiles[0][:, :512], tiles[0][:, 512:],
                                 tiles[1][:, :512], tiles[1][:, 512:]]
                        for dt in range(NDT):
                            for g in range(NG):
                                nc.tensor.matmul(
                                    slots[g],
                                    wsb[:, dt, pair * 128:(pair + 1) * 128],
                                    xtiles[dt][:, g * 512:(g + 1) * 512],
                                    start=(dt == 0), stop=(dt == NDT - 1),
                                )
                        for g in range(NG):
                            nc.scalar.activation(
                                dst[pair][:, g * 512:(g + 1) * 512],
                                slots[g],
                                AF.Identity, bias=bsb[:, pair:pair + 1],
                            )

            # ---- attention main loop ----
            with (
                tc.tile_pool(name="exps", bufs=2 * NJ + 2) as expp,
                tc.tile_pool(name="small", bufs=4) as small,
                tc.tile_pool(name="astage", bufs=6) as astage,
            ):
                def emit_normalize(g, pair, exps, psum_o):
                    J = 4 * g + 4
                    # late iterations run while ACT is mostly idle; early ones
                    # while ACT is saturated with exps -> pick evac engine
                    evac = nc.scalar.copy if g <= 1 else nc.vector.tensor_copy
                    rbs = {}
                    # both reciprocal chains first so neither's broadcast
                    # matmul queues behind the other's normalize TTs
                    for sub in range(2):
                        s_f = small.tile([1, 512], F32, tag="sf", name="s_f")
                        nc.vector.tensor_copy(s_f[:], psum_o[sub][HD:HD + 1, :])
                        r_f = small.tile([1, 512], F32, tag="rf", name="r_f")
                        nc.vector.reciprocal_approx_fast(r_f[:], s_f[:])
                        r_h = small.tile([1, 512], F16, tag="rh", name="r_h")
                        nc.vector.tensor_copy(r_h[:], r_f[:])
                        prb = ps.tile([128, 1024], F32, tag="mm", name="prb")
                        nc.tensor.matmul(
                            prb[:, :512], ones_row_h[:], r_h[:],
                            start=True, stop=True,
                        )
                        rb_sb = small.tile([128, 512], F16, tag="rb",
                                           name="rb_sb")
                        evac(rb_sb[:], prb[:, :512])
                        rbs[sub] = rb_sb
                    for sub in range(2):
                        off = 64 * sub
                        rb_sb = rbs[sub]
                        h = 2 * pair + sub
                        # 4 j-blocks share one staged tile and one (SWDGE)
                        # DMA; masked diagonal regions hold exact zeros
                        for jg in range(len(exps) // 4):
                            a4 = astage.tile([128, 4, 512], F16, tag="a",
                                             name="a4")
                            for jj in range(4):
                                j = 4 * jg + jj
                                nc.vector.tensor_tensor(
                                    a4[:, jj],
                                    exps[j][:, sub * 512:(sub + 1) * 512],
                                    rb_sb[:], MUL,
                                )
                            dst = attnt[
                                h, 512 * jg:512 * (jg + 1),
                                g * 512:(g + 1) * 512,
                            ].rearrange("(j p) w -> p j w", p=128)
                            nc.gpsimd.dma_start(dst, a4[:])
                        nc.vector.tensor_tensor(
                            oT_sb[pair][off:off + 64, g * 512:(g + 1) * 512],
                            psum_o[sub][:HD, :], rbs[sub][:HD, :], MUL,
                        )

                def emit_outproj(g):
                    evac = nc.scalar.copy if g <= 1 else nc.vector.tensor_copy
                    for mt in range(NDT):
                        pp = ps.tile([128, 1024], F32, tag="mm", name="pp")
                        for dt in range(2):
                            nc.tensor.matmul(
                                pp[:, :512],
                                wo_sb[:, dt, mt * 128:(mt + 1) * 128],
                                oT_sb[dt][:, g * 512:(g + 1) * 512],
                                start=(dt == 0), stop=(dt == 1),
                            )
                        o = astage.tile([128, 512], F32, tag="of", name="o")
                        evac(o[:], pp[:, :512])
                        nc.sync.dma_start(
                            outpt[mt * 128:(mt + 1) * 128,
                                  g * 512:(g + 1) * 512],
                            o[:],
                        )

                pending = None  # (g, pair, exps, psum_o) awaiting normalize
                for g in reversed(range(NG)):
                    J = 4 * g + 4
                    for pair in range(2):
                        exps = []  # e[j]: [128, 1024] = (sub0 | sub1) halves
                        pot = po.tile([128, 1024], F32, tag="o", name="pot")
                        psum_o = {
                            sub: pot[:HD + 1, 512 * sub:512 * sub + 512]
                            for sub in range(2)
                        }
                        for j0 in range(0, J, 2):
                            # each [128, 1024] PSUM pair-tile holds one k-tile
                            # j for BOTH heads of the pair (sub0 | sub1), so a
                            # single exp covers both with the right per-j
                            # padding bias. Consecutive score matmuls
                            # alternate base partitions 0/64 -> disjoint PE
                            # row groups run concurrently.
                            pss = {
                                jj: ps.tile([128, 1024], F32, tag="mm",
                                            name=f"pss{jj}")
                                for jj in range(2)
                            }
                            for jj in range(2):
                                j = j0 + jj
                                r = j - 4 * g
                                for sub in range(2):
                                    off = 64 * sub
                                    nc.tensor.matmul(
                                        pss[jj][:, sub * 512:(sub + 1) * 512],
                                        kT_sb[pair][off:off + 64,
                                                    j * 128:(j + 1) * 128],
                                        qT_sb[pair][off:off + 64,
                                                    g * 512:(g + 1) * 512],
                                        start=True, stop=(r < 0),
                                    )
                                if r >= 0:
                                    # staircase mask covers only the partial
                                    # 128-wide diagonal strip; the fully
                                    # masked columns below are never exp'd
                                    for sub in range(2):
                                        nc.tensor.matmul(
                                            pss[jj][:, sub * 512 + 128 * r:
                                                    sub * 512 + 128 * (r + 1)],
                                            negi_sb[:],
                                            masksq_sb[:],
                                            start=False, stop=True,
                                        )
                            for jj in range(2):
                                j = j0 + jj
                                r = j - 4 * g
                                w0 = 128 * r if r > 0 else 0
                                e = expp.tile([128, 1024], F16, tag="exp",
                                              name="e")
                                if w0 == 0:
                                    nc.scalar.activation(
                                        e[:], pss[jj][:], AF.Exp,
                                        bias=padb_sb[:, j:j + 1],
                                    )
                                else:
                                    for sub in range(2):
                                        nc.gpsimd.memset(
                                            e[:, sub * 512:sub * 512 + w0], 0.0
                                        )
                                        nc.scalar.activation(
                                            e[:, sub * 512 + w0:
                                              (sub + 1) * 512],
                                            pss[jj][:, sub * 512 + w0:
                                                    (sub + 1) * 512],
                                            AF.Exp,
                                            bias=padb_sb[:, j:j + 1],
                                        )
                                exps.append(e)
                            for jj in range(2):
                                j = j0 + jj
                                e = exps[j]
                                for sub in range(2):
                                    h = 2 * pair + sub
                                    nc.tensor.matmul(
                                        psum_o[sub][:],
                                        v_sb[j][:, h],
                                        e[:, sub * 512:(sub + 1) * 512],
                                        start=(j == 0), stop=(j == J - 1),
                                    )
                        # normalize of the PREVIOUS iteration lands here, so
                        # its reciprocal chain overlaps this iteration's
                        # matmul stream instead of stalling the PE
                        if pending is not None:
                            emit_normalize(*pending)
                            if pending[1] == 1:
                                emit_outproj(pending[0])
                        pending = (g, pair, exps, psum_o)
                emit_normalize(*pending)
                emit_outproj(pending[0])

    nc.compile()
    return nc


def _rearr_dxs(x, dtype=np.float16):
    # [Dm, S] -> [128, Dm//128, S] contiguous (partition-major d-tiles)
    return np.ascontiguousarray(
        x.reshape(x.shape[0] // 128, 128, x.shape[1]).transpose(1, 0, 2)
    ).astype(dtype)


def kernel(Q, K, V, attention_mask, wq, bq, wk, bk, wv, bv, wo, bo):
    Q = np.asarray(Q, np.float32)
    K = np.asarray(K, np.float32)
    V = np.asarray(V, np.float32)
    attention_mask = np.asarray(attention_mask)
    wq, bq_, wk, bk_ = (np.asarray(a, np.float32) for a in (wq, bq, wk, bk))
    wv, bv_, wo, bo_ = (np.asarray(a, np.float32) for a in (wv, bv, wo, bo))

    if "nc" not in _CACHED:
        _CACHED["nc"] = _build()
    nc = _CACHED["nc"]

    scale = 1.0 / np.sqrt(np.float32(HD))

    p = np.arange(128)[:, None]
    f = np.arange(128)[None, :]
    masksq = (f < p).astype(np.float16)  # invalid iff f' < p in diag strip
    negi = (NEG * np.eye(128)).astype(np.float16)

    in_maps = []
    for c in range(NCORES):
        b = c // (NCORES // B)
        hg = c % (NCORES // B)
        sl = slice(hg * HPC * HD, (hg + 1) * HPC * HD)  # this core's 256 dims

        padbias = np.where(attention_mask[b] != 0, 0.0, NEGPAD).astype(np.float32)
        in_maps.append({
            "qt": _rearr_dxs(Q[b].T),
            "kt": _rearr_dxs(K[b].T),
            "vt": _rearr_dxs(V[b].T),
            "wqt": _rearr_dxs(np.ascontiguousarray(wq.T[:, sl])),
            "wkt": _rearr_dxs(np.ascontiguousarray(wk.T[:, sl] * scale)),
            "wvt": _rearr_dxs(np.ascontiguousarray(wv.T[:, sl])),
            "wot": _rearr_dxs(np.ascontiguousarray(wo.T[sl, :])),
            "bq": np.ascontiguousarray(bq_[sl].reshape(2, 128).T),
            "bk": np.ascontiguousarray((bk_[sl] * scale).reshape(2, 128).T),
            "bvb": np.broadcast_to(bv_[sl], (128, 256)).copy(),
            "padb": np.ascontiguousarray(padbias.reshape(NJ, 128).T),
            "masksq": masksq,
            "negi": negi,
        })

    trace = bool(os.environ.get("MHA_TRACE"))
    res = run_bass_kernel_spmd(
        nc, in_maps, core_ids=list(range(NCORES)), trace=trace
    )
    if trace:
        kernel.last_exec_time_ns = res.exec_time_ns
        kernel.last_trace = (
            res.instructions_and_trace[1] if res.instructions_and_trace else None
        )

    # ---- host gather ----
    out = np.zeros((B, S, D), np.float32)
    attn = np.zeros((B, H, S, S), np.float32)
    tril = np.tril(np.ones((S, S), bool))
    for c in range(NCORES):
        b = c // (NCORES // B)
        hg = c % (NCORES // B)
        rc = res.results[c]
        out[b] += rc["outpt"].T
        for hl in range(HPC):
            h = hg * HPC + hl
            attn[b, h] = np.where(tril, rc["attnt"][hl].astype(np.float32).T, 0.0)
    out += bo_[None, None, :]
    return out, attn
